# revision 1
# baseline (speedup 1.0000x reference)
"""ConvSquare Trainium2 kernel.

Math: out = conv2d_3x3(x * poly(alpha), weight) + bias, stride 1, pad 1,
where poly(t) = (a*t + b)*t + c applied to the zero-padded alpha field.
(The reference's unfold/einsum collapses to this because x is zero-padded:
border window positions contribute x=0 regardless of the kernel value.)

Sharding: 8 cores = batch(4) x row-half(2). Each core computes a
[O=64, 64, 128] output slab from a zero-padded [C=64, 66, 130] input slab.

Tap pairing: y lives on SBUF partitions 0-63 (channels x full 66 padded
rows); partitions 64-127 hold y shifted DOWN one padded row (row r at
column f maps to row r+1). A single 128-contraction matmul then applies
taps (k=0,l) and (k=1,l) together: lhsT rows 0-63 = weight tap (0,l),
rows 64-127 = tap (1,l). The k=2 taps run as 64-contraction matmuls on
the lower half. 6 matmuls per 512-output chunk instead of 9.
"""

import sys

import numpy as np

sys.path.insert(0, "/opt/trn_rl_repo")

import concourse.bass as bass
import concourse.mybir as mybir
from concourse.bass_utils import run_bass_kernel_spmd
from concourse.tile import TileContext

F32 = mybir.dt.float32
F32R = mybir.dt.float32r

B, C, O, H, W = 4, 64, 64, 128, 128
HS = 64  # output rows per core
RP = HS + 2  # padded input rows (66)
WP = W + 2  # padded cols (130)
FREE = RP * WP  # 8580
SH_N = (RP - 1) * WP  # 8450: elements of the +1-row shifted copy
EW_CH = 6  # elementwise chunks (11 rows each)
EW_N = 11 * WP  # 1430
NCHUNK = 16  # matmul chunks (4 out rows each)
MM_N = 4 * W  # 512

_cache: dict = {}


def _program(av: float, bv: float, cv: float) -> bass.Bass:
    from concourse.bacc import Bacc

    nc = Bacc()
    x_h = nc.dram_tensor("x", [C, FREE], F32, kind="ExternalInput")
    al_h = nc.dram_tensor("al", [1, FREE], F32, kind="ExternalInput")
    w_h = nc.dram_tensor("w", [128, 384], F32R, kind="ExternalInput")
    bias_h = nc.dram_tensor("bias", [O, 1], F32, kind="ExternalInput")
    out_h = nc.dram_tensor("out", [O, HS * W], F32, kind="ExternalOutput")

    def mk_ap(base, offset, dims):
        return bass.AP(tensor=base.tensor, offset=offset, ap=dims)

    with TileContext(nc) as tc:
        with (
            tc.tile_pool(name="const", bufs=1) as cpool,
            tc.tile_pool(name="work", bufs=1) as wpool,
            tc.tile_pool(name="outs", bufs=4) as opool,
            tc.tile_pool(name="psum", bufs=8, space="PSUM") as ppool,
        ):
            wt = cpool.tile([128, 384], F32R)
            nc.sync.dma_start(out=wt[:, :], in_=w_h[:, :])
            bt = cpool.tile([O, 1], F32)
            nc.sync.dma_start(out=bt[:, :], in_=bias_h[:, :])

            xt = wpool.tile([64, FREE], F32)
            ab = wpool.tile([64, FREE], F32)
            tt = wpool.tile([64, FREE], F32)
            yt = wpool.tile([128, FREE], F32R)

            for j in range(EW_CH):
                sl = slice(j * EW_N, (j + 1) * EW_N)
                nc.sync.dma_start(out=xt[:, sl], in_=x_h[:, sl])
                nc.sync.dma_start(
                    out=ab[:, sl],
                    in_=mk_ap(al_h[:, :], j * EW_N, [[0, 64], [1, EW_N]]),
                )
                # t = a*alpha + b   (ACT engine)
                nc.scalar.activation(
                    tt[:, sl], ab[:, sl],
                    mybir.ActivationFunctionType.Copy, bias=bv, scale=av,
                )
                # t = t*alpha  (DVE)
                nc.vector.tensor_mul(tt[:, sl], tt[:, sl], ab[:, sl])
                # y = (t + c) * x  (DVE, rounds to f32r on write)
                nc.vector.scalar_tensor_tensor(
                    out=yt[0:64, sl], in0=tt[:, sl], scalar=cv, in1=xt[:, sl],
                    op0=mybir.AluOpType.add, op1=mybir.AluOpType.mult,
                )
                # +1-row shifted copy onto partitions 64-127 (SBUF->SBUF DMA).
                # copy range [j*EW_N - WP, (j+1)*EW_N - WP) reads exactly
                # chunk j's freshly written columns — no cross-chunk wait.
                c0 = max(0, j * EW_N - WP)
                c1 = min((j + 1) * EW_N - WP, SH_N)
                nc.sync.dma_start(
                    out=yt[64:128, c0:c1], in_=yt[0:64, c0 + WP:c1 + WP]
                )

            y3 = yt[:].rearrange("p (r c) -> p r c", r=RP)
            for i in range(NCHUNK):
                ps = ppool.tile([O, MM_N], F32)
                p3 = ps[:].rearrange("p (r c) -> p r c", r=4)
                # singles (k=2, lower half only) first: they don't need the
                # shifted copy, so PE can start before the copy DMA lands
                for l in range(3):
                    rhs_s = y3[0:64, 4 * i + 2:4 * i + 6, l:l + W]
                    lw_s = wt[0:64, 192 + 64 * l:192 + 64 * l + 64]
                    nc.tensor.matmul(
                        p3, lw_s, rhs_s, start=(l == 0), stop=False,
                    )
                for l in range(3):
                    # paired taps k=0 (lower half) + k=1 (shifted half)
                    rhs_p = y3[0:128, 4 * i:4 * i + 4, l:l + W]
                    lw_p = wt[0:128, 64 * l:64 * l + 64]
                    nc.tensor.matmul(
                        p3, lw_p, rhs_p, start=False, stop=(l == 2),
                    )
                ot = opool.tile([O, MM_N], F32)
                nc.vector.tensor_scalar(
                    out=ot[:, :], in0=ps[:, :], scalar1=bt[:, 0:1],
                    scalar2=None, op0=mybir.AluOpType.add,
                )
                nc.sync.dma_start(
                    out=out_h[:, 512 * i:512 * i + 512], in_=ot[:, :]
                )
    return nc


def _shard_inputs(x, alpha):
    """Per-core zero-padded slabs: x [C, 66*130], alpha [1, 66*130]."""
    maps = []
    for core in range(8):
        b_idx, h = divmod(core, 2)
        r0 = h * HS - 1  # global row of padded row 0
        xs = np.zeros((C, RP, WP), np.float32)
        als = np.zeros((1, RP, WP), np.float32)
        lo = max(0, r0)
        hi = min(H, r0 + RP)
        xs[:, lo - r0:hi - r0, 1:1 + W] = x[b_idx, :, lo:hi, :]
        als[:, lo - r0:hi - r0, 1:1 + W] = alpha[b_idx, :, lo:hi, :]
        maps.append({"x": xs.reshape(C, FREE), "al": als.reshape(1, FREE)})
    return maps


def _pack_weights(wt):
    """[O,C,3,3] -> [128, 384]: cols l*64+o rows c|c = taps (0,l)|(1,l);
    cols 192+l*64+o rows c (lower 64) = tap (2,l)."""
    wk = wt.transpose(1, 2, 3, 0)  # [c, k, l, o]
    pair = np.concatenate([wk[:, 0], wk[:, 1]], axis=0).reshape(128, 192)
    single = wk[:, 2].reshape(64, 192)
    out = np.zeros((128, 384), np.float32)
    out[:, :192] = pair
    out[:64, 192:] = single
    return np.ascontiguousarray(out)


def kernel(inputs, alpha, weight, bias, a, b, c):
    x = np.ascontiguousarray(np.asarray(inputs, np.float32))
    al = np.ascontiguousarray(np.asarray(alpha, np.float32))
    wt = np.asarray(weight, np.float32)
    bs = np.asarray(bias, np.float32)
    av, bv, cv = float(a), float(b), float(c)

    key = (av, bv, cv)
    if key not in _cache:
        _cache.clear()
        nc_new = _program(av, bv, cv)
        nc_new.finalize()
        _cache[key] = nc_new
    nc = _cache[key]

    w_packed = _pack_weights(wt)
    b_packed = np.ascontiguousarray(bs.reshape(O, 1))
    in_maps = _shard_inputs(x, al)
    for m in in_maps:
        m["w"] = w_packed
        m["bias"] = b_packed

    res = run_bass_kernel_spmd(nc, in_maps, list(range(8)))

    out = np.empty((B, O, H, W), np.float32)
    for core in range(8):
        b_idx, h = divmod(core, 2)
        out[b_idx, :, h * HS:(h + 1) * HS, :] = res.results[core]["out"].reshape(
            O, HS, W
        )
    return out



# revision 9
# speedup vs baseline: 1.2595x; 1.2595x over previous
"""ConvSquare Trainium2 kernel.

Math: out = conv2d_3x3(x * poly(alpha), weight) + bias, stride 1, pad 1,
where poly(t) = (a*t + b)*t + c applied to the zero-padded alpha field.
(The reference's unfold/einsum collapses to this because x is zero-padded:
border window positions contribute x=0 regardless of the kernel value.)

Sharding: 8 cores = batch(4) x row-half(2). Each core computes a
[O=64, 64, 128] output slab from a zero-padded [C=64, 66, 130] input slab.

v2 design (vs the f32 baseline):
- bf16 data path: x, kal, y, weights, out are bf16 (DMA bytes halved,
  matmul still 1 cycle/row). Output converted back to f32 on host.
- poly(alpha) computed on a [66,130] tile (partition=row) so the
  elementwise cost is ~130 free-rows instead of 3 passes over [64,8580];
  the result is flattened to [1,8580] and DMA-broadcast to 64 partitions.
- y = x * kal via scalar_tensor_tensor (all-bf16 SBUF operands -> 4x DVE).
- bias-add + PSUM->SBUF moves to the ACT engine (Identity + bias AP),
  leaving DVE nearly idle.
- tap pairing as before: partitions 64-127 hold y shifted down one padded
  row; 3 paired matmuls (k=0,1) + 3 single matmuls (k=2) per 512-out chunk.
- PE warmup matmuls during the prologue so the p-state ramp (~3us at
  half speed) is spent on throwaway work.
"""

import sys

import ml_dtypes
import numpy as np

sys.path.insert(0, "/opt/trn_rl_repo")

import concourse.bass as bass
import concourse.mybir as mybir
from concourse.bass_utils import run_bass_kernel_spmd
from concourse.tile import TileContext

F32 = mybir.dt.float32
BF16 = mybir.dt.bfloat16
NPBF16 = ml_dtypes.bfloat16

B, C, O, H, W = 4, 64, 64, 128, 128
HS = 64  # output rows per core
RP = HS + 2  # padded input rows (66)
WP = W + 2  # padded cols (130)
FREE = RP * WP  # 8580
SH_N = (RP - 1) * WP  # 8450: elements of the +1-row shifted copy
# elementwise chunks by padded-row ranges (small first for fast pipeline start)
EW_ROWS = [(0, 11), (11, 33), (33, 66)]
NCHUNK = 16  # matmul chunks (4 out rows each)
MM_N = 4 * W  # 512
OG = 4  # output DMA groups (4 psum chunks each)

_cache: dict = {}


def _program(av: float, bv: float, cv: float) -> bass.Bass:
    from concourse.bacc import Bacc

    nc = Bacc()
    x_h = nc.dram_tensor("x", [C, FREE], BF16, kind="ExternalInput")
    al_h = nc.dram_tensor("al", [RP, WP], F32, kind="ExternalInput")
    w_h = nc.dram_tensor("w", [128, 384], BF16, kind="ExternalInput")
    bias_h = nc.dram_tensor("bias", [O, 1], F32, kind="ExternalInput")
    out_h = nc.dram_tensor("out", [O, HS * W], BF16, kind="ExternalOutput")
    kfl_h = nc.dram_tensor("kfl", [1, FREE], BF16, kind="Internal")

    def mk_ap(base, offset, dims):
        return bass.AP(tensor=base.tensor, offset=offset, ap=dims)

    with TileContext(nc) as tc:
        with (
            tc.tile_pool(name="const", bufs=1) as cpool,
            tc.tile_pool(name="work", bufs=1) as wpool,
            tc.tile_pool(name="outs", bufs=2) as opool,
            tc.tile_pool(name="psum", bufs=7, space="PSUM") as ppool,
            tc.tile_pool(name="wpsum", bufs=1, space="PSUM") as wppool,
        ):
            wt = cpool.tile([128, 384], BF16)
            nc.sync.dma_start(out=wt[:, :], in_=w_h[:, :])
            bt = cpool.tile([O, 1], F32)
            nc.sync.dma_start(out=bt[:, :], in_=bias_h[:, :])
            alt = cpool.tile([RP, WP], F32)
            nc.sync.dma_start(out=alt[:, :], in_=al_h[:, :])

            # PE warmup on garbage-but-finite weight data: burn the p-state
            # ramp on throwaway matmuls while the alpha/x DMAs land.
            wps = wppool.tile([O, 384], F32)
            for _ in range(8):
                nc.tensor.matmul(
                    wps[:, :], wt[0:128, 0:64], wt[0:128, 0:384],
                    start=True, stop=True,
                )

            # poly(alpha) on the [66,130] tile: kal = (a*al + b)*al + c
            pt = cpool.tile([RP, WP], F32)
            nc.scalar.activation(
                pt[:, :], alt[:, :],
                mybir.ActivationFunctionType.Copy, bias=bv, scale=av,
            )
            nc.vector.tensor_mul(pt[:, :], pt[:, :], alt[:, :])
            kal = cpool.tile([RP, WP], BF16)
            nc.scalar.activation(
                kal[:, :], pt[:, :],
                mybir.ActivationFunctionType.Copy, bias=cv, scale=1.0,
            )
            # flatten [66,130] -> DRAM [1, 8580] (stride-0 partition
            # broadcast is only legal from DRAM sources)
            nc.sync.dma_start(out=kfl_h[0:1, :], in_=kal[:, :])

            xt = wpool.tile([64, FREE], BF16)
            kb = wpool.tile([64, FREE], BF16)
            yt = wpool.tile([128, FREE], BF16)

            # staged input DMAs: x and kal-broadcast per row-chunk
            for (r0, r1) in EW_ROWS:
                sl = slice(r0 * WP, r1 * WP)
                nc.sync.dma_start(out=xt[:, sl], in_=x_h[:, sl])
                nc.sync.dma_start(
                    out=kb[:, sl],
                    in_=mk_ap(kfl_h[:, :], r0 * WP, [[0, 64], [1, (r1 - r0) * WP]]),
                )
            for j, (r0, r1) in enumerate(EW_ROWS):
                sl = slice(r0 * WP, r1 * WP)
                # y = x * kal  (DVE 4x mode: all-bf16 SBUF operands)
                nc.vector.scalar_tensor_tensor(
                    out=yt[0:64, sl], in0=xt[:, sl], scalar=0.0, in1=kb[:, sl],
                    op0=mybir.AluOpType.add, op1=mybir.AluOpType.mult,
                )
                # +1-row shifted copy onto partitions 64-127 (SBUF->SBUF DMA):
                # dest rows r0-1..r1-1 read exactly this chunk's rows r0..r1.
                c0 = max(0, r0 * WP - WP)
                c1 = r1 * WP - WP
                nc.sync.dma_start(
                    out=yt[64:128, c0:c1], in_=yt[0:64, c0 + WP:c1 + WP]
                )

            y3 = yt[:].rearrange("p (r c) -> p r c", r=RP)
            ot = None
            for i in range(NCHUNK):
                g, gi = divmod(i, NCHUNK // OG)
                if gi == 0:
                    ot = opool.tile([O, (NCHUNK // OG) * MM_N], BF16)
                ps = ppool.tile([O, MM_N], F32)
                p3 = ps[:].rearrange("p (r c) -> p r c", r=4)
                # singles (k=2, lower half only) first: they don't need the
                # shifted copy, so PE can start before the copy DMA lands
                for l in range(3):
                    rhs_s = y3[0:64, 4 * i + 2:4 * i + 6, l:l + W]
                    lw_s = wt[0:64, 192 + 64 * l:192 + 64 * l + 64]
                    nc.tensor.matmul(
                        p3, lw_s, rhs_s, start=(l == 0), stop=False,
                    )
                for l in range(3):
                    # paired taps k=0 (lower half) + k=1 (shifted half)
                    rhs_p = y3[0:128, 4 * i:4 * i + 4, l:l + W]
                    lw_p = wt[0:128, 64 * l:64 * l + 64]
                    nc.tensor.matmul(
                        p3, lw_p, rhs_p, start=False, stop=(l == 2),
                    )
                # bias add + downcast on ACT engine (Identity allows bias AP)
                nc.scalar.activation(
                    ot[:, gi * MM_N:(gi + 1) * MM_N], ps[:, :],
                    mybir.ActivationFunctionType.Identity,
                    bias=bt[:, 0:1], scale=1.0,
                )
                if gi == NCHUNK // OG - 1:
                    o0 = (g * (NCHUNK // OG)) * MM_N
                    nc.sync.dma_start(
                        out=out_h[:, o0:o0 + (NCHUNK // OG) * MM_N], in_=ot[:, :]
                    )
    return nc


def _shard_inputs(x, alpha):
    """Per-core zero-padded slabs: x bf16 [C, 66*130], alpha f32 [66, 130]."""
    maps = []
    for core in range(8):
        b_idx, h = divmod(core, 2)
        r0 = h * HS - 1  # global row of padded row 0
        xs = np.zeros((C, RP, WP), NPBF16)
        als = np.zeros((RP, WP), np.float32)
        lo = max(0, r0)
        hi = min(H, r0 + RP)
        xs[:, lo - r0:hi - r0, 1:1 + W] = x[b_idx, :, lo:hi, :].astype(NPBF16)
        als[lo - r0:hi - r0, 1:1 + W] = alpha[b_idx, 0, lo:hi, :]
        maps.append({"x": xs.reshape(C, FREE), "al": als})
    return maps


def _pack_weights(wt):
    """[O,C,3,3] -> [128, 384] bf16: cols l*64+o rows c|c = taps (0,l)|(1,l);
    cols 192+l*64+o rows c (lower 64) = tap (2,l)."""
    wk = wt.transpose(1, 2, 3, 0)  # [c, k, l, o]
    pair = np.concatenate([wk[:, 0], wk[:, 1]], axis=0).reshape(128, 192)
    single = wk[:, 2].reshape(64, 192)
    out = np.zeros((128, 384), np.float32)
    out[:, :192] = pair
    out[:64, 192:] = single
    return np.ascontiguousarray(out.astype(NPBF16))


def kernel(inputs, alpha, weight, bias, a, b, c):
    x = np.ascontiguousarray(np.asarray(inputs, np.float32))
    al = np.ascontiguousarray(np.asarray(alpha, np.float32))
    wt = np.asarray(weight, np.float32)
    bs = np.asarray(bias, np.float32)
    av, bv, cv = float(a), float(b), float(c)

    key = (av, bv, cv)
    if key not in _cache:
        _cache.clear()
        nc_new = _program(av, bv, cv)
        nc_new.finalize()
        _cache[key] = nc_new
    nc = _cache[key]

    w_packed = _pack_weights(wt)
    b_packed = np.ascontiguousarray(bs.reshape(O, 1))
    in_maps = _shard_inputs(x, al)
    for m in in_maps:
        m["w"] = w_packed
        m["bias"] = b_packed

    res = run_bass_kernel_spmd(nc, in_maps, list(range(8)))

    out = np.empty((B, O, H, W), np.float32)
    for core in range(8):
        b_idx, h = divmod(core, 2)
        out[b_idx, :, h * HS:(h + 1) * HS, :] = (
            res.results[core]["out"].astype(np.float32).reshape(O, HS, W)
        )
    return out


# revision 10
# speedup vs baseline: 1.2831x; 1.0187x over previous
"""ConvSquare Trainium2 kernel.

Math: out = conv2d_3x3(x * poly(alpha), weight) + bias, stride 1, pad 1,
where poly(t) = (a*t + b)*t + c applied to the zero-padded alpha field.
(The reference's unfold/einsum collapses to this because x is zero-padded:
border window positions contribute x=0 regardless of the kernel value.)

Sharding: 8 cores = batch(4) x row-half(2). Each core computes a
[O=64, 64, 128] output slab from a zero-padded [C=64, 66, 130] input slab.

v3 design:
- bf16 matmul path (x, y, weights, out), fp16 alpha/poly path (alpha in
  [0,1]; fp16 keeps the kernel-field rounding ~0.05% where bf16 would be
  ~0.4%). Output converted back to f32 on host.
- alpha is broadcast from DRAM per chunk (stride-0 partition DMA) and the
  poly is evaluated in broadcast form: ACT t=a*ab+b, DVE u=t*ab (2x mode),
  kal=u+c (4x mode), y=x*kal (2x mode). No small-tile detour, no DRAM
  bounce: shortest serial chain to the first matmul.
- 5 matmuls per 512-col output chunk (vs 9 naive): partitions 64-127 of
  Y1 hold y shifted down one padded row (pairs taps (0,l)+(1,l)); a second
  tile Y2 holds [y; y shifted one column] (pairs taps (2,0)+(2,1)); plus
  one single matmul for tap (2,2).
- replica copies (row-shift, y copy, col-shift) issue from the Pool
  engine's SWDGE queue; inputs/outputs from SP's HWDGE queue - two DMA
  queues, no head-of-line blocking.
- bias-add + PSUM->SBUF downcast on the ACT engine (Identity + bias AP).
- output DMA groups of [6,6,3,1] chunks so the last transfer is small.
"""

import sys

import ml_dtypes
import numpy as np

sys.path.insert(0, "/opt/trn_rl_repo")

import concourse.bass as bass
import concourse.mybir as mybir
from concourse.bass_utils import run_bass_kernel_spmd
from concourse.tile import TileContext

F32 = mybir.dt.float32
BF16 = mybir.dt.bfloat16
FP16 = mybir.dt.float16
NPBF16 = ml_dtypes.bfloat16

B, C, O, H, W = 4, 64, 64, 128, 128
HS = 64  # output rows per core
RP = HS + 2  # padded input rows (66)
WP = W + 2  # padded cols (130)
FREE = RP * WP  # 8580
SH_N = (RP - 1) * WP  # 8450: elements of the +1-row shifted copy
EW_ROWS = [(0, 6), (6, 16), (16, 38), (38, 66)]
NCHUNK = 16  # matmul chunks (4 out rows each)
MM_N = 4 * W  # 512
OGROUPS = [6, 6, 3, 1]  # output DMA groups (psum chunks each)

_cache: dict = {}


def _program(av: float, bv: float, cv: float) -> bass.Bass:
    from concourse.bacc import Bacc

    nc = Bacc()
    x_h = nc.dram_tensor("x", [C, FREE], BF16, kind="ExternalInput")
    alf_h = nc.dram_tensor("alf", [1, FREE], FP16, kind="ExternalInput")
    w_h = nc.dram_tensor("w", [128, 320], BF16, kind="ExternalInput")
    bias_h = nc.dram_tensor("bias", [O, 1], F32, kind="ExternalInput")
    out_h = nc.dram_tensor("out", [O, HS * W], BF16, kind="ExternalOutput")

    def mk_ap(base, offset, dims):
        return bass.AP(tensor=base.tensor, offset=offset, ap=dims)

    with TileContext(nc) as tc:
        with (
            tc.tile_pool(name="const", bufs=1) as cpool,
            tc.tile_pool(name="work", bufs=1) as wpool,
            tc.tile_pool(name="outs", bufs=2) as opool,
            tc.tile_pool(name="psum", bufs=8, space="PSUM") as ppool,
        ):
            wt = cpool.tile([128, 320], BF16)
            bt = cpool.tile([O, 1], F32)
            ab = wpool.tile([64, FREE], FP16)
            ts = wpool.tile([64, FREE], FP16)
            kb = wpool.tile([64, FREE], FP16)
            xt = wpool.tile([64, FREE], BF16)
            y1 = wpool.tile([128, FREE], BF16)
            y2 = wpool.tile([128, FREE], BF16)

            # SP/HWDGE queue: inputs (interleaved per chunk), weights, bias
            for j, (r0, r1) in enumerate(EW_ROWS):
                sl = slice(r0 * WP, r1 * WP)
                nc.sync.dma_start(
                    out=ab[:, sl],
                    in_=mk_ap(alf_h[:, :], r0 * WP, [[0, 64], [1, (r1 - r0) * WP]]),
                )
                nc.sync.dma_start(out=xt[:, sl], in_=x_h[:, sl])
                if j == 0:
                    nc.sync.dma_start(out=wt[:, :], in_=w_h[:, :])
                    nc.sync.dma_start(out=bt[:, :], in_=bias_h[:, :])

            # per-chunk poly + y, then replica copies on the Pool queue
            for j, (r0, r1) in enumerate(EW_ROWS):
                sl = slice(r0 * WP, r1 * WP)
                # t = a*alpha + b  (ACT)
                nc.scalar.activation(
                    ts[:, sl], ab[:, sl],
                    mybir.ActivationFunctionType.Copy, bias=bv, scale=av,
                )
                # u = t*alpha  (DVE 2x)
                nc.vector.tensor_mul(ts[:, sl], ts[:, sl], ab[:, sl])
                # kal = u + c  (DVE 4x)
                nc.vector.tensor_scalar(
                    out=kb[:, sl], in0=ts[:, sl], scalar1=cv, scalar2=None,
                    op0=mybir.AluOpType.add,
                )
                # y = x * kal  (DVE 2x)
                nc.vector.tensor_mul(y1[0:64, sl], xt[:, sl], kb[:, sl])
                # Pool/SWDGE queue: row-shift into y1 upper, y copy + col-shift
                # into y2 (all read exactly this chunk's freshly written cols)
                a0, b0 = r0 * WP, r1 * WP
                c0, c1 = max(0, a0 - WP), b0 - WP
                nc.gpsimd.dma_start(
                    out=y1[64:128, c0:c1], in_=y1[0:64, c0 + WP:c1 + WP]
                )
                nc.gpsimd.dma_start(out=y2[0:64, a0:b0], in_=y1[0:64, a0:b0])
                d0, d1 = max(0, a0 - 1), b0 - 1
                nc.gpsimd.dma_start(
                    out=y2[64:128, d0:d1], in_=y1[0:64, d0 + 1:d1 + 1]
                )

            y1_3 = y1[:].rearrange("p (r c) -> p r c", r=RP)
            y2_3 = y2[:].rearrange("p (r c) -> p r c", r=RP)
            gi0 = 0
            ot = None
            gsz = None
            g = 0
            for i in range(NCHUNK):
                if i == gi0:
                    gsz = OGROUPS[g]
                    ot = opool.tile([O, gsz * MM_N], BF16)
                ps = ppool.tile([O, MM_N], F32)
                p3 = ps[:].rearrange("p (r c) -> p r c", r=4)
                # single tap (2,2): lower y only - ready first
                nc.tensor.matmul(
                    p3, wt[0:64, 0:64], y1_3[0:64, 4 * i + 2:4 * i + 6, 2:2 + W],
                    start=True, stop=False,
                )
                # row pairs (0,l)+(1,l)
                for l in range(3):
                    nc.tensor.matmul(
                        p3, wt[0:128, 64 + 64 * l:128 + 64 * l],
                        y1_3[0:128, 4 * i:4 * i + 4, l:l + W],
                        start=False, stop=False,
                    )
                # col pair (2,0)+(2,1) on y2
                nc.tensor.matmul(
                    p3, wt[0:128, 256:320],
                    y2_3[0:128, 4 * i + 2:4 * i + 6, 0:W],
                    start=False, stop=True,
                )
                # bias add + downcast on ACT engine
                oc = (i - gi0) * MM_N
                nc.scalar.activation(
                    ot[:, oc:oc + MM_N], ps[:, :],
                    mybir.ActivationFunctionType.Identity,
                    bias=bt[:, 0:1], scale=1.0,
                )
                if i - gi0 == gsz - 1:
                    nc.sync.dma_start(
                        out=out_h[:, gi0 * MM_N:(gi0 + gsz) * MM_N], in_=ot[:, :]
                    )
                    gi0 += gsz
                    g += 1
    return nc


def _shard_inputs(x, alpha):
    """Per-core zero-padded slabs: x bf16 [C, 66*130], alpha fp16 [1, 66*130]."""
    maps = []
    for core in range(8):
        b_idx, h = divmod(core, 2)
        r0 = h * HS - 1  # global row of padded row 0
        xs = np.zeros((C, RP, WP), NPBF16)
        als = np.zeros((RP, WP), np.float16)
        lo = max(0, r0)
        hi = min(H, r0 + RP)
        xs[:, lo - r0:hi - r0, 1:1 + W] = x[b_idx, :, lo:hi, :].astype(NPBF16)
        als[lo - r0:hi - r0, 1:1 + W] = alpha[b_idx, 0, lo:hi, :]
        maps.append({"x": xs.reshape(C, FREE), "alf": als.reshape(1, FREE)})
    return maps


def _pack_weights(wt):
    """[O,C,3,3] -> [128, 320] bf16.
    cols 0:64        rows 0:64  = tap (2,2)          (single)
    cols 64+64l:+64  rows c|64+c = taps (0,l)|(1,l)  (row pairs)
    cols 256:320     rows c|64+c = taps (2,0)|(2,1)  (col pair)
    """
    wk = wt.transpose(1, 2, 3, 0)  # [c, k, l, o]
    out = np.zeros((128, 320), np.float32)
    out[:64, 0:64] = wk[:, 2, 2]
    for l in range(3):
        out[:64, 64 + 64 * l:128 + 64 * l] = wk[:, 0, l]
        out[64:, 64 + 64 * l:128 + 64 * l] = wk[:, 1, l]
    out[:64, 256:320] = wk[:, 2, 0]
    out[64:, 256:320] = wk[:, 2, 1]
    return np.ascontiguousarray(out.astype(NPBF16))


def kernel(inputs, alpha, weight, bias, a, b, c):
    x = np.ascontiguousarray(np.asarray(inputs, np.float32))
    al = np.ascontiguousarray(np.asarray(alpha, np.float32))
    wt = np.asarray(weight, np.float32)
    bs = np.asarray(bias, np.float32)
    av, bv, cv = float(a), float(b), float(c)

    key = (av, bv, cv)
    if key not in _cache:
        _cache.clear()
        nc_new = _program(av, bv, cv)
        nc_new.finalize()
        _cache[key] = nc_new
    nc = _cache[key]

    w_packed = _pack_weights(wt)
    b_packed = np.ascontiguousarray(bs.reshape(O, 1))
    in_maps = _shard_inputs(x, al)
    for m in in_maps:
        m["w"] = w_packed
        m["bias"] = b_packed

    res = run_bass_kernel_spmd(nc, in_maps, list(range(8)))

    out = np.empty((B, O, H, W), np.float32)
    for core in range(8):
        b_idx, h = divmod(core, 2)
        out[b_idx, :, h * HS:(h + 1) * HS, :] = (
            res.results[core]["out"].astype(np.float32).reshape(O, HS, W)
        )
    return out
